# revision 2
# baseline (speedup 1.0000x reference)
"""Trainium2 Bass kernel for CentroidTDBase.compute_dparams (B=256, D=256, F=512).

Math (p=0.05, m=0.95, s=0.05/B, KC=1/(B*F)):
    Szz  = z^T z ; dmu = s*Szz - 0.05*mu ; mu_t = mu + dmu
    M    = (p^2 - mu_t) * Szz                       (elementwise)
    dkf^T[i,d] = q_i*(XzT[i,d]/B - wfT[i,d]*zbar_i) + wfT[i,d]*r_i - KC*(M @ w_f^T)[i,d]
      with q = p - diag(mu_t), r_i = KC * sum_j M[i,j]
    dbias = q ;  dkp = u_prev^T td / B

Sharding: each of the 8 cores owns a 64-wide block of the F axis ("i" index of
dmu/dkf/dbias rows); dkernel_p is 2D-sharded (4x2 grid of 128x256 blocks).
All contractions stay on-core -> no collectives. Per-core inputs are
host-sliced/packed, outputs host-concatenated.

Device-side structure per core (kernel is identical across cores; only the
host-packed data differs):
  - z is column-permuted so the owned block comes first (zb = p1[:, 0:64]).
  - P = s*Szz rows via 2 fp32r matmuls (N=512, 1 cyc/row after rounding copies).
  - [zbar | zsqsum]/B and XzT/B via fp32 matmuls against xa = [x/B | 1/B].
  - M rows on DVE (half-split for pipelining), transposed on the PE (via an
    on-device identity), evacuated to bf16, contracted with bf16 w_f^T*KS.
  - The wfTb*(q*zbar - r) rank-1-scaled term is folded into the same PSUM
    accumulation group as a diag(a2) matmul, so the tail is one subtract.
  - ~26 dependency-free identity transposes warm the PE clock (HAM) while
    the first DMAs are in flight.
  - dkp matmuls (fp32, N=256) are emitted last; outputs go out as three DMAs
    (dmu early, dkp, then the small [dbias|dkfT] pack).
"""

import threading

import numpy as np
import ml_dtypes

import concourse.bass as bass
import concourse.tile as tile
from concourse import bacc, mybir
from concourse.masks import make_identity
from concourse.bass_utils import run_bass_kernel_spmd

F32 = mybir.dt.float32
F32R = mybir.dt.float32r
BF16 = mybir.dt.bfloat16
AF = mybir.ActivationFunctionType
ALU = mybir.AluOpType

B, D, F, FB = 256, 256, 512, 64
NCORES = 8
P_TGT, MOM = 0.05, 0.95
S = (1.0 - MOM) / B
KC = 1.0 / (B * F)
P2 = P_TGT * P_TGT
KS = KC / S

_lock = threading.Lock()
_compiled = None


def _build(ndummy=26):
    nc = bacc.Bacc("TRN2", target_bir_lowering=False, debug=False)
    p1_d = nc.dram_tensor("p1", [B, 512], F32, kind="ExternalInput")
    p2_d = nc.dram_tensor("p2", [FB, 769], F32, kind="ExternalInput")
    p3_d = nc.dram_tensor("p3", [B, 257], F32, kind="ExternalInput")
    p4_d = nc.dram_tensor("p4", [B, 384], F32, kind="ExternalInput")
    p5_d = nc.dram_tensor("p5", [F, D], BF16, kind="ExternalInput")
    out_d = nc.dram_tensor("out_pack", [FB, 769], F32, kind="ExternalOutput")
    dkp_d = nc.dram_tensor("dkp2", [128, D], F32, kind="ExternalOutput")

    with tile.TileContext(nc) as tc:
        with (
            tc.tile_pool(name="sb", bufs=1) as sb,
            tc.tile_pool(name="ps", bufs=1, space="PSUM") as ps,
        ):
            p1_sb = sb.tile([128, 2, 512], F32)
            p2_sb = sb.tile([FB, 769], F32)
            p3_sb = sb.tile([128, 2, 257], F32)
            p4_sb = sb.tile([128, 2, 384], F32)
            p5_sb = sb.tile([128, 4, D], BF16)

            nc.sync.dma_start(p1_sb[:, 0, :], p1_d[0:128, :])
            nc.sync.dma_start(p1_sb[:, 1, :], p1_d[128:256, :])
            nc.sync.dma_start(p2_sb[:], p2_d[:])
            nc.sync.dma_start(p3_sb[:], p3_d[:].rearrange("(t p) f -> p t f", p=128))
            nc.sync.dma_start(p4_sb[:], p4_d[:].rearrange("(t p) f -> p t f", p=128))
            nc.sync.dma_start(p5_sb[:], p5_d[:].rearrange("(c p) d -> p c d", p=128))

            ident = sb.tile([FB, FB], F32)
            make_identity(nc, ident[:])

            dummy_ps = ps.tile([FB, FB], F32, tag="dummy")
            for _ in range(ndummy):
                nc.tensor.transpose(dummy_ps[:], ident[:], ident[:])

            # fp32r operands for the Szz matmul
            p1_r = sb.tile([128, 2, 512], F32R)
            zbs_r = sb.tile([128, 2, FB], F32R)
            zbsq = sb.tile([128, 2, FB], F32)
            for t in range(2):
                nc.vector.tensor_copy(p1_r[:, t, :], p1_sb[:, t, :])
                nc.scalar.activation(zbs_r[:, t, :], p1_sb[:, t, 0:FB], AF.Copy, scale=S)
            nc.vector.tensor_mul(zbsq[:], p1_sb[:, :, 0:FB], p1_sb[:, :, 0:FB])

            g_ap = p2_sb[:, 0:512]
            h_ap = p2_sb[:, 512:513]
            wfTb_ap = p2_sb[:, 513:769]

            # 0.05*mu rows = (p^2 - g)/19
            tmp05 = sb.tile([FB, 512], F32)
            nc.gpsimd.tensor_scalar(tmp05[:], g_ap, -1.0 / 19.0, P2 / 19.0, ALU.mult, ALU.add)

            szz_ps = ps.tile([FB, 512], F32, tag="szz")
            xz_ps = ps.tile([FB, 257], F32, tag="xz")
            zsq_ps = ps.tile([FB, 1], F32, tag="zsq")
            dkp_ps = ps.tile([128, D], F32, tag="kp")
            ct_ps = ps.tile([FB, D], F32, tag="ct")

            for t in range(2):
                nc.tensor.matmul(szz_ps[:], zbs_r[:, t, :], p1_r[:, t, :],
                                 start=(t == 0), stop=(t == 1))
            for t in range(2):
                nc.tensor.matmul(xz_ps[:], p1_sb[:, t, 0:FB], p3_sb[:, t, :],
                                 start=(t == 0), stop=(t == 1))
            for t in range(2):
                nc.tensor.matmul(zsq_ps[:], zbsq[:, t, :], p3_sb[:, t, 256:257],
                                 start=(t == 0), stop=(t == 1))

            # M chain, half-split so transposes can start on the first half
            p_sb = sb.tile([FB, 512], F32)
            m1 = sb.tile([FB, 512], F32)
            mt = sb.tile([FB, 512], F32)
            out_sb = sb.tile([FB, 769], F32)
            for hh in range(2):
                sl = slice(hh * 256, (hh + 1) * 256)
                nc.scalar.activation(p_sb[:, sl], szz_ps[:, sl], AF.Copy)
                nc.vector.tensor_sub(m1[:, sl], g_ap[:, sl], p_sb[:, sl])
                nc.vector.tensor_mul(mt[:, sl], m1[:, sl], p_sb[:, sl])
            nc.gpsimd.tensor_sub(out_sb[:, 0:512], p_sb[:], tmp05[:])  # dmu rows (perm)

            r_red = sb.tile([FB, 1], F32)
            nc.vector.tensor_reduce(r_red[:], mt[:], mybir.AxisListType.X, ALU.add)

            mtT = sb.tile([128, 4, FB], BF16)
            for c in range(4):
                tr_ps = ps.tile([128, FB], F32, name=f"tr{c}", tag="tr", bufs=2)
                nc.tensor.transpose(tr_ps[:], mt[:, c * 128:(c + 1) * 128], ident[:])
                nc.scalar.activation(mtT[:, c, :], tr_ps[:], AF.Copy)

            for c in range(4):
                nc.tensor.matmul(ct_ps[:], mtT[:, c, :], p5_sb[:, c, :],
                                 start=(c == 0), stop=False)

            # combine scalars (xz-gated, early)
            zbar = sb.tile([FB, 1], F32)
            t2 = sb.tile([FB, 1], F32)
            qz = sb.tile([FB, 1], F32)
            a2 = sb.tile([FB, 1], F32)
            diag_a2 = sb.tile([FB, FB], F32)
            e1 = sb.tile([FB, D], F32)
            dkp_sb = sb.tile([128, D], F32)

            nc.scalar.activation(zbar[:], xz_ps[:, 256:257], AF.Copy)
            nc.scalar.activation(t2[:], zsq_ps[:], AF.Copy, scale=0.05)
            nc.vector.tensor_sub(out_sb[:, 512:513], h_ap, t2[:])  # q = dbias
            q_ap = out_sb[:, 512:513]
            nc.vector.tensor_mul(qz[:], q_ap, zbar[:])
            nc.scalar.activation(e1[:], xz_ps[:, 0:256], AF.Copy, scale=q_ap)
            # a2 = q*zbar - KS*rowsum(mt)
            nc.vector.tensor_scalar(a2[:], r_red[:], -KS, qz[:], ALU.mult, ALU.add)
            nc.gpsimd.tensor_scalar_mul(diag_a2[:], ident[:], a2[:])
            # fold wfTb*a2 into the CT accumulation; then dkfT = e1 - ct
            nc.tensor.matmul(ct_ps[:], diag_a2[:], wfTb_ap, start=False, stop=True)
            for t in range(2):
                nc.tensor.matmul(dkp_ps[:], p4_sb[:, t, 0:128], p4_sb[:, t, 128:384],
                                 start=(t == 0), stop=(t == 1))
            nc.vector.tensor_sub(out_sb[:, 513:769], e1[:], ct_ps[:])  # dkfT
            nc.scalar.activation(dkp_sb[:], dkp_ps[:], AF.Copy)

            nc.sync.dma_start(out_d[:, 0:512], out_sb[:, 0:512])
            nc.sync.dma_start(dkp_d[:], dkp_sb[:])
            nc.sync.dma_start(out_d[:, 512:769], out_sb[:, 512:769])
    nc.compile()
    return nc


def _get_compiled():
    global _compiled
    with _lock:
        if _compiled is None:
            _compiled = _build()
    return _compiled


def _perm(k):
    blk = np.arange(k * FB, (k + 1) * FB)
    rest = np.concatenate([np.arange(0, k * FB), np.arange((k + 1) * FB, F)])
    return np.concatenate([blk, rest])


def _make_in_maps(z, x, u_prev, td, mu, wf):
    z = np.ascontiguousarray(z, np.float32)
    x = np.ascontiguousarray(x, np.float32)
    u_prev = np.ascontiguousarray(u_prev, np.float32)
    td = np.ascontiguousarray(td, np.float32)
    mu = np.ascontiguousarray(mu, np.float32)
    wf = np.ascontiguousarray(wf, np.float32)
    wfT = np.ascontiguousarray(wf.T)
    diag_mu = np.diag(mu)
    xa = np.concatenate([x / B, np.full((B, 1), 1.0 / B, np.float32)], axis=1)
    in_maps = []
    for k in range(NCORES):
        perm = _perm(k)
        blk = perm[0:FB]
        rk, ck = k // 2, k % 2
        g = (P2 - MOM * mu[k * FB:(k + 1) * FB, :][:, perm]).astype(np.float32)
        h = (P_TGT - MOM * diag_mu[blk]).reshape(FB, 1).astype(np.float32)
        p2p = np.concatenate([g, h, wfT[blk, :]], axis=1)
        p4 = np.concatenate([u_prev[:, 128 * rk:128 * (rk + 1)] / B,
                             td[:, 256 * ck:256 * (ck + 1)]], axis=1)
        in_maps.append({
            "p1": np.ascontiguousarray(z[:, perm], np.float32),
            "p2": np.ascontiguousarray(p2p, np.float32),
            "p3": np.ascontiguousarray(xa, np.float32),
            "p4": np.ascontiguousarray(p4, np.float32),
            "p5": np.ascontiguousarray((wfT[perm, :] * KS).astype(ml_dtypes.bfloat16)),
        })
    return in_maps


def _unpack(results):
    dmu = np.zeros((F, F), np.float32)
    dkp = np.zeros((F, F), np.float32)
    dkfT_rows, dbias_rows = [], []
    for k in range(NCORES):
        out = results[k]["out_pack"]
        dmu[k * FB:(k + 1) * FB, _perm(k)] = out[:, 0:512]
        dbias_rows.append(out[:, 512])
        dkfT_rows.append(out[:, 513:769])
        rk, ck = k // 2, k % 2
        dkp[128 * rk:128 * (rk + 1), 256 * ck:256 * (ck + 1)] = results[k]["dkp2"]
    dkfT = np.concatenate(dkfT_rows, axis=0)
    dbias = np.ascontiguousarray(np.concatenate(dbias_rows, axis=0), dtype=np.float32)
    return (np.ascontiguousarray(dmu), np.ascontiguousarray(dkfT.T, dtype=np.float32),
            dbias, np.ascontiguousarray(dkp), dbias.copy())


def _run(in_maps, **kwargs):
    return run_bass_kernel_spmd(_get_compiled(), in_maps, core_ids=list(range(NCORES)), **kwargs)


def kernel(z, x, u_prev, td_pred_err_prev, mu, w_f_kernel, w_p_kernel=None, **_ignored):
    in_maps = _make_in_maps(z, x, u_prev, td_pred_err_prev, mu, w_f_kernel)
    res = _run(in_maps).results
    return _unpack(res)
